# revision 6
# baseline (speedup 1.0000x reference)
"""Trainium2 Bass kernel v2 for nn_Captioner_41412074668572 (retrieval_knn).

Computes: mean over (b, n) of min over l of ||image_features[b,n] - emb_table[token_ids[b,l]]||_2

v2 strategy (vs v1's out[n,l] / x-stationary / 512 weight loads):
  out[l, n] layout with y STATIONARY (32 weight loads/core instead of 512),
  x streams as the 512-wide moving operand -> matmul runs at the wide-free-dim
  production rate; fp8 DoubleRow (K=256/matmul) doubles PE throughput.

  d2[l, n] = -2*y.x (PE, fp8 DoubleRow) + x2[n] (PE, K=2 bf16 hi/lo aug matmul)
             + y2[l] (ACT per-partition bias, exact f32)
  cost[l, n] = sqrt(d2) on ACT (pre-min: sqrt is monotone, d2 ~ 1700 >> 0)
  min over l (partition axis) via DVE: 32x32 block transpose -> per-32-segment
  free-axis min -> 2 partition-fold min ops -> [32, 64] distances per batch
  -> row-sum -> acc[32, B_LOC] -> host sums and divides.

Sharding: data-parallel over batch B=32 -> 4 batches/core on 8 cores.
"""

import numpy as np
import ml_dtypes

B, N, L, D, V = 32, 2048, 128, 1024, 32000
N_CORES = 8
B_LOC = B // N_CORES          # 4 batches per core
P = 128                       # partitions
NCH = 4                       # 512-wide n-chunks per batch (PSUM bank width)
CW = N // NCH                 # 512

_CACHE: dict = {}

BF16 = ml_dtypes.bfloat16
FP8 = ml_dtypes.float8_e4m3


DEFAULT_KNOBS = dict(
    dr=True,          # fp8 DoubleRow (K=256/matmul); False -> plain K=128 (bf16 rate)
    x_split=1,        # DMA splits per x k-chunk
    fp16_sc=True,     # sqrt output fp16 (False -> bf16)
    y_eng="sync",     # engine for y/y2/x2a DMAs
    x_bufs=4,         # x tile buffering depth
    x_eng="sync",     # "sync"=all x on sync ring; "both"=alternate sync/scalar
    tail_eng="gpsimd",  # ring for tail fold DMAs (keeps HWDGE FIFOs clean)
    m1_bufs=2,        # m1all double buffering across reps
    aux_bufs=4,       # y/x2a/y2b tile buffering
    x_one=True,       # single 2MB x DMA per batch (partition-major DRAM layout)
    x2_fold="tail",   # "mm": K=2 aug matmul; "tail": x2+sqrt after the folds
)


def _build_nc(reps: int = 1, **knobs):
    import concourse.tile as tile
    from concourse import bacc, mybir

    kn = dict(DEFAULT_KNOBS)
    kn.update(knobs)

    f32 = mybir.dt.float32
    bf16 = mybir.dt.bfloat16
    fp16 = mybir.dt.float16 if kn["fp16_sc"] else mybir.dt.bfloat16
    fp8 = mybir.dt.float8e4

    DR = kn["dr"]
    KC = 4 if DR else 8       # contraction chunks (256 or 128 wide)
    KJ = 2 if DR else 1       # k-tiles per chunk (DoubleRow interleave)

    nc = bacc.Bacc("TRN2", target_bir_lowering=False, debug=False,
                   num_devices=N_CORES)

    # DRAM inputs (per-core shards, laid out by make_in_maps)
    if kn["x_one"]:
        x8 = nc.dram_tensor("x8", [B_LOC, P, KC, KJ, N], fp8, kind="ExternalInput")
    else:
        x8 = nc.dram_tensor("x8", [B_LOC, KC, P, KJ, N], fp8, kind="ExternalInput")
    y8 = nc.dram_tensor("y8", [P, B_LOC, KC, KJ, L], fp8, kind="ExternalInput")
    if kn["x2_fold"] == "mm":
        x2a = nc.dram_tensor("x2a", [B_LOC, 2, N], bf16, kind="ExternalInput")
    else:
        x2f = nc.dram_tensor("x2f", [32, B_LOC * 64], f32, kind="ExternalInput")
    y2b = nc.dram_tensor("y2b", [P, B_LOC], f32, kind="ExternalInput")
    out = nc.dram_tensor("out", [32, 1], f32, kind="ExternalOutput")

    with tile.TileContext(nc) as tc:
        with (
            tc.tile_pool(name="xp", bufs=2) as xp,
            tc.tile_pool(name="yp", bufs=2) as yp,
            tc.tile_pool(name="aux", bufs=2) as aux,
            tc.tile_pool(name="cons", bufs=1) as cons,
            tc.tile_pool(name="scp", bufs=2) as scp,
            tc.tile_pool(name="post", bufs=2) as post,
            tc.tile_pool(name="ps", bufs=2, space="PSUM") as pp,
        ):
            if kn["x2_fold"] == "mm":
                ones2 = cons.tile([2, P], bf16, tag="ones2")
                nc.gpsimd.memset(ones2[:], 1.0)
            y_eng = getattr(nc, kn["y_eng"])

            def emit_rep_head():
                # per-rep loads: all 4 batches' y + y2 in one DMA each.
                # bufs=3: with 2, the rep-head trigger waits on the previous
                # rep's matmuls and that wait blocks the sync FIFO.
                yta = yp.tile([P, B_LOC, KC, KJ, L], fp8, tag="y", bufs=3)
                y_eng.dma_start(yta[:], y8[:])
                y2ta = aux.tile([P, B_LOC], f32, tag="y2", bufs=3)
                y_eng.dma_start(y2ta[:], y2b[:])
                return yta, y2ta

            def emit_batch(b, m1all, yta, y2ta):
                yt = yta[:, b]
                y2t = y2ta[:, b:b + 1]
                # ---- DMAs ----
                xt = xp.tile([P, KC, KJ, N], fp8, tag="x", bufs=kn["x_bufs"])
                if kn["x_one"]:
                    nc.sync.dma_start(xt[:], x8[b])
                else:
                    xs = kn["x_split"]
                    w = N // xs
                    for kc in range(KC):
                        for s in range(xs):
                            if kn["x_eng"] == "both" and (kc * xs + s) % 2:
                                eng = nc.scalar
                            else:
                                eng = nc.sync
                            eng.dma_start(xt[:, kc, :, s * w:(s + 1) * w],
                                          x8[b, kc][:, :, s * w:(s + 1) * w])
                if kn["x2_fold"] == "mm":
                    x2t = aux.tile([2, N], bf16, tag="x2", bufs=kn["aux_bufs"])
                    y_eng.dma_start(x2t[:], x2a[b])

                # ---- matmuls: ps[c] = -2*y.x (+ x2) ----
                pss = [pp.tile([P, CW], f32, tag=f"ps{c}", name=f"ps{c}")[:]
                       for c in range(NCH)]
                for kc in range(KC):
                    for c in range(NCH):
                        if DR:
                            nc.tensor.matmul(
                                pss[c],
                                yt[:, kc],                       # [128, 2, 128]
                                xt[:, kc, :, c * CW:(c + 1) * CW],  # [128, 2, 512]
                                start=(kc == 0),
                                stop=(kc == KC - 1) and kn["x2_fold"] != "mm",
                                perf_mode=mybir.MatmulPerfMode.DoubleRow,
                            )
                        else:
                            nc.tensor.matmul(
                                pss[c],
                                yt[:, kc, 0],                    # [128, 128]
                                xt[:, kc, 0, c * CW:(c + 1) * CW],  # [128, 512]
                                start=(kc == 0),
                                stop=(kc == KC - 1) and kn["x2_fold"] != "mm",
                            )
                if kn["x2_fold"] == "mm":
                    for c in range(NCH):
                        nc.tensor.matmul(
                            pss[c], ones2[:],
                            x2t[:, c * CW:(c + 1) * CW],
                            start=False, stop=True,
                        )

                # ---- ACT: sc = sqrt(ps+y2) (mm) or ps+y2-1024 (tail) ----
                sc = scp.tile([P, N], fp16, tag="sc")
                act_fn = (mybir.ActivationFunctionType.Sqrt
                          if kn["x2_fold"] == "mm"
                          else mybir.ActivationFunctionType.Identity)
                for c in range(NCH):
                    nc.scalar.activation(
                        sc[:, c * CW:(c + 1) * CW], pss[c],
                        func=act_fn,
                        bias=y2t[:], scale=1.0,
                    )

                # ---- DVE: fused 32x32 block transpose + 32-seg min ----
                nc.vector.tensor_reduce(
                    m1all[:, b * 64:(b + 1) * 64],
                    sc[:].rearrange("p (a b) -> p a b", b=32),
                    axis=mybir.AxisListType.X, op=mybir.AluOpType.min,
                    apply_transpose=True,
                )

            def emit_tail(m1all):
                # fold the 4 partition quadrants (l-blocks); DVE can't mix
                # partition bases, so shift halves via SBUF DMA.
                FW = B_LOC * 64
                t_eng = getattr(nc, kn["tail_eng"])
                m1b = post.tile([64, FW], f32, tag="m1b")
                t_eng.dma_start(m1b[:], m1all[64:128, :])
                f1 = post.tile([64, FW], f32, tag="f1")
                nc.vector.tensor_tensor(f1[:], m1all[0:64, :], m1b[:],
                                        op=mybir.AluOpType.min)
                f1b = post.tile([32, FW], f32, tag="f1b")
                t_eng.dma_start(f1b[:], f1[32:64, :])
                f2 = post.tile([32, FW], f32, tag="f2")
                nc.vector.tensor_tensor(f2[:], f1[0:32, :], f1b[:],
                                        op=mybir.AluOpType.min)
                if kn["x2_fold"] != "mm":
                    # f2 holds min(-2xy + y2) - 1024; add x2+1024, sqrt, sum
                    x2ft = aux.tile([32, FW], f32, tag="x2f", bufs=2)
                    y_eng.dma_start(x2ft[:], x2f[:])
                    m2 = post.tile([32, FW], f32, tag="m2")
                    nc.vector.tensor_add(m2[:], f2[:], x2ft[:])
                    sq = post.tile([32, FW], f32, tag="sq")
                    nc.scalar.sqrt(sq[:], m2[:])
                    f2 = sq
                ov = cons.tile([32, 1], f32, tag="ov", bufs=2)
                nc.vector.reduce_sum(ov[:], f2[:], axis=mybir.AxisListType.X)
                t_eng.dma_start(out[:], ov[:])

            # software-pipelined: rep r's tail is emitted after rep r+1's
            # first batch so its chain never blocks the ACT/DVE rings at
            # the rep boundary.
            pending = None
            for _ in range(reps):
                m1all = cons.tile([P, B_LOC * 64], f32, tag="m1all",
                                  bufs=kn["m1_bufs"], name="m1all")
                yta, y2ta = emit_rep_head()
                for b in range(B_LOC):
                    emit_batch(b, m1all, yta, y2ta)
                    if b == 0 and pending is not None:
                        emit_tail(pending)
                        pending = None
                pending = m1all
            emit_tail(pending)

    nc.compile()
    return nc


def _get_nc(reps: int = 1, **knobs):
    key = ("nc", reps, tuple(sorted(knobs.items())))
    if key not in _CACHE:
        _CACHE[key] = _build_nc(reps, **knobs)
    return _CACHE[key]


def make_in_maps(image_features: np.ndarray, token_ids: np.ndarray,
                 emb_table: np.ndarray, **knobs) -> list[dict]:
    """Shard + lay out the full inputs into per-core device input maps."""
    kn = dict(DEFAULT_KNOBS)
    kn.update(knobs)
    DR = kn["dr"]
    KC = 4 if DR else 8
    KJ = 2 if DR else 1
    assert kn == dict(DEFAULT_KNOBS) or True

    x = np.asarray(image_features, dtype=np.float32)
    tok = np.asarray(token_ids)
    emb = np.asarray(emb_table, dtype=np.float32)

    in_maps = []
    for c in range(N_CORES):
        xc = x[c * B_LOC:(c + 1) * B_LOC]                       # [4, N, D]
        # x8[b, kc, p, j, n] = x[b, n, kc*(128*KJ) + j*128 + p]
        xT = np.ascontiguousarray(xc.transpose(0, 2, 1))        # [4, D, N]
        if kn["x_one"]:
            # [b, p, kc, j, n]
            x8_dev = np.ascontiguousarray(
                xT.reshape(B_LOC, KC, KJ, P, N).transpose(0, 3, 1, 2, 4)
            ).astype(FP8)
        else:
            x8_dev = np.ascontiguousarray(
                xT.reshape(B_LOC, KC, KJ, P, N).transpose(0, 1, 3, 2, 4)
            ).astype(FP8)

        x2 = np.square(xc).sum(axis=-1, dtype=np.float64).astype(np.float32)
        x2_hi = x2.astype(BF16)
        x2_lo = (x2 - x2_hi.astype(np.float32)).astype(BF16)
        x2a_dev = np.ascontiguousarray(np.stack([x2_hi, x2_lo], axis=1))  # [4,2,N]
        # tail layout: x2f[i, b*64 + j] = x2[b, 32*j + i] + 1024 (centering undo)
        x2f_dev = np.ascontiguousarray(
            (x2 + 1024.0).reshape(B_LOC, 64, 32).transpose(2, 0, 1)
            .reshape(32, B_LOC * 64))

        y = emb[tok[c * B_LOC:(c + 1) * B_LOC]]                 # [4, L, D]
        yT = np.ascontiguousarray((-2.0 * y).transpose(0, 2, 1))  # [4, D, L]
        # y8[p, b, kc, j, l] = -2y[b, l, kc*(128*KJ) + j*128 + p]
        y8_dev = np.ascontiguousarray(
            yT.reshape(B_LOC, KC, KJ, P, L).transpose(3, 0, 1, 2, 4)
        ).astype(FP8)

        y2 = np.square(y).sum(axis=-1, dtype=np.float64)        # [4, L]
        y2off = 0.0 if kn["x2_fold"] == "mm" else 1024.0
        y2b_dev = np.ascontiguousarray(
            (y2 - y2off).astype(np.float32).T)                  # [128, 4]

        in_maps.append({
            "x8": x8_dev,
            "y8": y8_dev,
            "x2a": x2a_dev,
            "x2f": x2f_dev,
            "y2b": y2b_dev,
        })
    return in_maps


def kernel(image_features: np.ndarray, token_ids: np.ndarray,
           emb_table: np.ndarray) -> np.ndarray:
    from concourse import mybir
    from concourse.bass_utils import run_bass_kernel_spmd

    nc = _get_nc()
    declared = {
        alloc.memorylocations[0].name
        for alloc in nc.m.functions[0].allocations
        if isinstance(alloc, mybir.MemoryLocationSet)
        and alloc.kind == "ExternalInput"
    }
    in_maps = [
        {k: v for k, v in m.items() if k in declared}
        for m in make_in_maps(image_features, token_ids, emb_table)
    ]
    res = run_bass_kernel_spmd(nc, in_maps, core_ids=list(range(N_CORES)))
    total = np.float64(0.0)
    for c in range(N_CORES):
        total += res.results[c]["out"].astype(np.float64).sum()
    return np.float32(total / (B * N))
